# revision 1
# baseline (speedup 1.0000x reference)
"""Bass/Tile TRN2 kernel for CrossAttention (B=2, N=4096, D=512, H=8, DH=64).

Sharding: batch*heads over 8 cores — core c handles batch c//4 and heads
(c%4)*2, (c%4)*2+1. Each core computes its two heads' attention and the
partial output projection O_h @ Wo_h; the host sums the 4 partials per batch.

Per-core dataflow (one NeuronCore, Tile-scheduled; per-block tiles so the
projection phase overlaps the attention phase):
  xT [512,4096] (host-pretransposed x[b]) -> SBUF per 512-column block
  Qt,Kt [128,512] per block = W^T x^T   (heads stacked: h0 = partitions 0:64)
  V natural [128,130] per 128-row j-chunk as [V_h0 | 1 | V_h1 | 1]
  per (i-block 512, j-chunk 128):
     St = Kt^T Qt                  (row-packed 2 heads -> one 2-bank PSUM tile)
     Pt = exp(SCALE*St)            (ScalarE [128,1024] call; no max-sub:
                                    logits are O(1) for this problem family)
     O' += [V|1]^T Pt              (PSUM accum; row 64 = softmax denominator)
  epilogue: rinv = 1/O'[64] (DVE), transpose rinv to partitions via K=1 fp32
  matmul against ones, project unnormalized O with Wo (row-packed), scale the
  two head partials by rinv as per-partition scalars, sum + bias, DMA out.
"""

import sys

if "/opt/trn_rl_repo" not in sys.path:
    sys.path.insert(0, "/opt/trn_rl_repo")

import numpy as np

B, N, D = 2, 4096, 512
H, DH = 8, 64
SCALE = DH ** -0.5
P = 128
IB = 512            # i/column block
NDC = D // P        # 4 contraction chunks for projections
NIB = N // IB       # 8
NJC = N // P        # 32 key chunks
NQ = IB // P        # 4 out-proj chunks per i-block

_CACHE: dict = {}


def _build(n_attn_ib=NIB):
    import concourse.mybir as mybir
    from concourse import bacc
    from concourse.tile import TileContext

    f32 = mybir.dt.float32
    f32r = mybir.dt.float32r
    Exp = mybir.ActivationFunctionType.Exp

    nc = bacc.Bacc("TRN2")
    xT = nc.dram_tensor("xT", [D, N], f32r, kind="ExternalInput")
    wq = nc.dram_tensor("wq", [D, 2 * DH], f32r, kind="ExternalInput")
    wk = nc.dram_tensor("wk", [D, 2 * DH], f32r, kind="ExternalInput")
    wv = nc.dram_tensor("wv", [D, 2 * DH], f32r, kind="ExternalInput")
    wo = nc.dram_tensor("wo", [2 * DH, D], f32r, kind="ExternalInput")
    bo = nc.dram_tensor("bo", [D], f32, kind="ExternalInput")
    out = nc.dram_tensor("out", [N, D], f32, kind="ExternalOutput")

    with TileContext(nc) as tc, \
         tc.tile_pool(name="persist", bufs=1) as pp:
        # per-block persistent SBUF tensors (separate tiles => fine deps)
        xtb = [pp.tile([P, NDC, IB], f32r, name=f"xt{i}", tag=f"xt{i}")
               for i in range(NIB)]
        qtb = [pp.tile([P, IB], f32r, name=f"qt{i}", tag=f"qt{i}")
               for i in range(NIB)]
        ktb = [pp.tile([P, IB], f32r, name=f"kt{i}", tag=f"kt{i}")
               for i in range(NIB)]
        vtb = [pp.tile([P, NQ, 130], f32r, name=f"vt{i}", tag=f"vt{i}")
               for i in range(NIB)]
        wq_sb = pp.tile([P, NDC, 2 * DH], f32r, name="wq_sb", tag="wq")
        wk_sb = pp.tile([P, NDC, 2 * DH], f32r, name="wk_sb", tag="wk")
        # wv padded with wq columns to a 256-wide moving operand: fp32r
        # matmuls only hit full rate at free dim >= 256 (cols 128:256 unused)
        wv_sb = pp.tile([P, NDC, 4 * DH], f32r, name="wv_sb", tag="wv")
        wo_sb = pp.tile([P, D], f32r, name="wo_sb", tag="wo")
        bo_sb = pp.tile([1, D], f32, name="bo_sb", tag="bos")
        bo_bc = pp.tile([P, D], f32, name="bo_bc", tag="bob")
        one_sb = pp.tile([1, 1], f32, name="one_sb", tag="one")

        for dc in range(NDC):
            nc.sync.dma_start(wq_sb[:, dc, :], wq[dc * P:(dc + 1) * P, :])
            nc.sync.dma_start(wk_sb[:, dc, :], wk[dc * P:(dc + 1) * P, :])
            nc.sync.dma_start(wv_sb[:, dc, 0:2 * DH],
                              wv[dc * P:(dc + 1) * P, :])
            nc.sync.dma_start(wv_sb[:, dc, 2 * DH:4 * DH],
                              wq[dc * P:(dc + 1) * P, :])
        nc.sync.dma_start(wo_sb[:], wo[:, :])
        nc.sync.dma_start(bo_sb[:], bo[None, :])
        nc.gpsimd.partition_broadcast(bo_bc[:], bo_sb[:])
        nc.vector.memset(one_sb[:], 1.0)
        for ibb in range(NIB):
            nc.vector.memset(vtb[ibb][:, :, 64:65].bitcast(f32), 1.0)
            nc.vector.memset(vtb[ibb][:, :, 129:130].bitcast(f32), 1.0)

        with tc.tile_pool(name="ps", bufs=2, space="PSUM") as ps_pool, \
             tc.tile_pool(name="po", bufs=2, space="PSUM") as po_pool, \
             tc.tile_pool(name="pe", bufs=1, space="PSUM") as pe_pool, \
             tc.tile_pool(name="pt", bufs=6) as pt_pool, \
             tc.tile_pool(name="ep", bufs=3) as ep_pool, \
             tc.tile_pool(name="ot", bufs=6) as ot_pool:

            for ibb in range(NIB):
                for dc in range(NDC):
                    nc.sync.dma_start(xtb[ibb][:, dc, :],
                                      xT[dc * P:(dc + 1) * P,
                                         ibb * IB:(ibb + 1) * IB])

            def phase_a_block(ibb):
                """Project column block ibb's K, V, Q."""
                xt = xtb[ibb]
                for dst, w_sb in ((ktb[ibb], wk_sb), (qtb[ibb], wq_sb)):
                    pq = ps_pool.tile([P, IB], f32, tag="st", name="pq")
                    for dc in range(NDC):
                        nc.tensor.matmul(pq[:], w_sb[:, dc, :], xt[:, dc, :],
                                         start=(dc == 0), stop=(dc == NDC - 1))
                    nc.vector.tensor_copy(dst[:], pq[:])
                for q in range(NQ):
                    pv = ps_pool.tile([P, 4 * DH], f32, tag="st", name="pv")
                    for dc in range(NDC):
                        nc.tensor.matmul(
                            pv[:], xt[:, dc, q * P:(q + 1) * P],
                            wv_sb[:, dc, :],
                            start=(dc == 0), stop=(dc == NDC - 1))
                    nc.vector.tensor_copy(vtb[ibb][:, q, 0:DH], pv[:, 0:DH])
                    nc.vector.tensor_copy(vtb[ibb][:, q, 65:65 + DH],
                                          pv[:, DH:2 * DH])

            # ---- attention (phase A for block b fused before its first use) ----
            for ib in range(n_attn_ib):
                qt = qtb[ib]
                o0 = po_pool.tile([65, IB], f32, tag="o", name="o0")
                o1 = po_pool.tile([65, IB], f32, tag="o", name="o1")
                for jc in range(NJC):
                    if ib == 0 and jc % NQ == 0:
                        phase_a_block(jc // NQ)
                    kt = ktb[jc // NQ]
                    k0 = (jc % NQ) * P
                    st = ps_pool.tile([P, 2 * IB], f32, tag="st", name="st")
                    nc.tensor.matmul(st[:, 0:IB],
                                     kt[0:DH, k0:k0 + P], qt[0:DH, :],
                                     start=True, stop=True,
                                     tile_position=(0, 0))
                    nc.tensor.matmul(st[:, IB:2 * IB],
                                     kt[DH:P, k0:k0 + P], qt[DH:P, :],
                                     start=True, stop=True,
                                     tile_position=(64, 0))
                    pt = pt_pool.tile([P, 2 * IB], f32r, tag="pt", name="pt")
                    nc.scalar.activation(pt[:], st[:], Exp, scale=SCALE)
                    vt = vtb[jc // NQ][:, jc % NQ, :]
                    nc.tensor.matmul(o0[:], vt[:, 0:65], pt[:, 0:IB],
                                     start=(jc == 0), stop=(jc == NJC - 1))
                    nc.tensor.matmul(o1[:], vt[:, 65:130], pt[:, IB:2 * IB],
                                     start=(jc == 0), stop=(jc == NJC - 1))
                # epilogue: denominators -> per-partition scalars via K=1 fp32
                # matmul transpose; project unnormalized; scale, sum, bias.
                rinv0 = ep_pool.tile([1, IB], f32, tag="rinv", name="rinv0")
                rinv1 = ep_pool.tile([1, IB], f32, tag="rinv", name="rinv1")
                nc.vector.reciprocal(rinv0[:], o0[64:65, :])
                nc.vector.reciprocal(rinv1[:], o1[64:65, :])
                rtp = pe_pool.tile([P, 2 * NQ], f32, tag="ep", name="rtp")
                for q in range(NQ):
                    nc.tensor.matmul(rtp[:, q:q + 1],
                                     rinv0[0:1, q * P:(q + 1) * P], one_sb[:],
                                     start=True, stop=True)
                    nc.tensor.matmul(rtp[:, NQ + q:NQ + q + 1],
                                     rinv1[0:1, q * P:(q + 1) * P], one_sb[:],
                                     start=True, stop=True)
                rts = ep_pool.tile([P, 2 * NQ], f32, tag="rts", name="rts")
                nc.vector.tensor_copy(rts[:], rtp[:])
                ots = ep_pool.tile([P, IB], f32r, tag="otn", name="ots")
                nc.vector.tensor_copy(ots[0:DH, :], o0[0:DH, :])
                nc.vector.tensor_copy(ots[DH:P, :], o1[0:DH, :])
                for q in range(NQ):
                    q0, q1 = q * P, (q + 1) * P
                    ppx = pe_pool.tile([P, 2 * D], f32, tag="ep", name="ppx")
                    nc.tensor.matmul(ppx[:, 0:D], ots[0:DH, q0:q1],
                                     wo_sb[0:DH, :],
                                     start=True, stop=True,
                                     tile_position=(0, 0))
                    nc.tensor.matmul(ppx[:, D:2 * D], ots[DH:P, q0:q1],
                                     wo_sb[DH:P, :],
                                     start=True, stop=True,
                                     tile_position=(64, 0))
                    t0 = ot_pool.tile([P, D], f32, tag="t0", name="t0")
                    t1 = ot_pool.tile([P, D], f32, tag="t1", name="t1")
                    nc.vector.tensor_scalar_mul(t0[:], ppx[:, 0:D],
                                                rts[:, q:q + 1])
                    nc.vector.tensor_scalar_mul(t1[:], ppx[:, D:2 * D],
                                                rts[:, NQ + q:NQ + q + 1])
                    otile = ot_pool.tile([P, D], f32, tag="out", name="otile")
                    nc.gpsimd.tensor_add(otile[:], t0[:], t1[:])
                    nc.gpsimd.tensor_add(otile[:], otile[:], bo_bc[:])
                    nc.sync.dma_start(out[ib * IB + q0:ib * IB + q1, :],
                                      otile[:])

    nc.compile()
    return nc


def _get_nc():
    if "nc" not in _CACHE:
        _CACHE["nc"] = _build()
    return _CACHE["nc"]


def kernel(x, Wq, Wk, Wv, Wo, bo):
    from concourse.bass_utils import run_bass_kernel_spmd

    x = np.asarray(x, dtype=np.float32)
    Wq = np.asarray(Wq, dtype=np.float32)
    Wk = np.asarray(Wk, dtype=np.float32)
    Wv = np.asarray(Wv, dtype=np.float32)
    Wo = np.asarray(Wo, dtype=np.float32)
    bo = np.asarray(bo, dtype=np.float32)

    nc = _get_nc()

    xTs = [np.ascontiguousarray(x[b].T) for b in range(B)]
    zeros_bo = np.zeros_like(bo)
    in_maps = []
    for c in range(8):
        b, p = c // 4, c % 4
        sl = slice(p * 2 * DH, (p + 1) * 2 * DH)
        in_maps.append({
            "xT": xTs[b],
            "wq": np.ascontiguousarray(Wq[:, sl]),
            "wk": np.ascontiguousarray(Wk[:, sl]),
            "wv": np.ascontiguousarray(Wv[:, sl]),
            "wo": np.ascontiguousarray(Wo[sl, :]),
            "bo": bo if p == 0 else zeros_bo,
        })

    try:
        res = run_bass_kernel_spmd(nc, in_maps, core_ids=list(range(8)))
    except Exception:
        # transient device wedge (NRT_EXEC_UNIT_UNRECOVERABLE) — retry once
        import time as _time
        _time.sleep(45)
        res = run_bass_kernel_spmd(nc, in_maps, core_ids=list(range(8)))
    parts = [res.results[c]["out"] for c in range(8)]
    full = np.stack([
        parts[0] + parts[1] + parts[2] + parts[3],
        parts[4] + parts[5] + parts[6] + parts[7],
    ]).astype(np.float32)
    return full



# revision 2
# speedup vs baseline: 1.0123x; 1.0123x over previous
"""Bass/Tile TRN2 kernel v5 for CrossAttention (B=2, N=4096, D=512, H=8, DH=64).

Sharding: batch*heads over 8 cores — core c handles batch c//4 and heads
(c%4)*2, (c%4)*2+1.

Design (cost-model driven):
  - S = K^T Q in fp8e4 DoubleRow (0.5 cycles/row). The two DoubleRow k-tiles
    carry (Q_hi, Q_lo) — a double-fp8 split of Q — against duplicated fp8 K,
    so only K's fp8 rounding reaches the logits (~1e-2 Frobenius).
  - exp split across engines: 20/32 key-chunks on ACT (exp(S*SCALE)), 12/32
    via GPSIMD pow (DVE stages S to SBUF fp16, Pool computes (e^SCALE)**S).
  - O = [V|1]^T P in fp16; denominator rides as V's 65th column.
  - Projections (phase A) interleave with the first query block, sharing the
    S-tile PSUM slots; S pipeline runs with 3-chunk lookahead.
  - Per-block epilogue is split so the PE never starves: at the block
    boundary only two DVE copies drain the O accumulators (freeing their
    PSUM bank for the next block); the normalization (reciprocal, a PE
    broadcast-matmul, DVE multiplies) and the output projection run spread
    over the next block.
"""

import sys

if "/opt/trn_rl_repo" not in sys.path:
    sys.path.insert(0, "/opt/trn_rl_repo")

import numpy as np

B, N, D = 2, 4096, 512
H, DH = 8, 64
SCALE = DH ** -0.5
P = 128
IB = 512            # i/query block
NDC = D // P        # 4 contraction chunks for projections
NIB = N // IB       # 8
NJC = N // P        # 32 key chunks per query block
LA = 3              # S-matmul chunks in flight (= st pool bufs)

# key-chunk indices handled by the GPSIMD pow path (rest go to ACT exp)
def is_pool(jc):
    return jc % 3 == 1  # 11 of 32
# extra O-emission lag for pool-path chunks (their exp chain is longer)
POOL_LAG = 4

_CACHE: dict = {}


def _build():
    import concourse.mybir as mybir
    from concourse import bacc
    from concourse.tile import TileContext

    f32 = mybir.dt.float32
    f32r = mybir.dt.float32r
    fp16 = mybir.dt.float16
    fp8 = mybir.dt.float8e4
    Exp = mybir.ActivationFunctionType.Exp
    DR = mybir.MatmulPerfMode.DoubleRow
    Pow = mybir.AluOpType.pow
    Mult = mybir.AluOpType.mult
    Sub = mybir.AluOpType.subtract

    nc = bacc.Bacc("TRN2")
    # host pre-chunks contraction-dim tiles: [128, NDC, ...]
    xT = nc.dram_tensor("xT", [P, NDC, N], fp16, kind="ExternalInput")
    wq = nc.dram_tensor("wq", [P, NDC, 2 * DH], fp16, kind="ExternalInput")
    wk = nc.dram_tensor("wk", [P, NDC, 2 * DH], fp16, kind="ExternalInput")
    wv = nc.dram_tensor("wv", [P, NDC, 2 * DH], fp16, kind="ExternalInput")
    # augmented output projection: [Wo_h0; bo | Wo_h1; 0], shape [DH+1, 2*D]
    wo = nc.dram_tensor("wo", [DH + 1, 2 * D], f32r, kind="ExternalInput")
    out = nc.dram_tensor("out", [N, D], f32, kind="ExternalOutput")

    with TileContext(nc) as tc, \
         tc.tile_pool(name="persist", bufs=1) as pp:
        xtb = [pp.tile([P, NDC, IB], fp16, name=f"xt{i}", tag=f"xt{i}")
               for i in range(NIB)]
        # Q per block: [128 (2 heads x 64 d), 2 k-tiles, 512] = (Q_hi, Q_lo)
        qtb = [pp.tile([P, 2, IB], fp8, name=f"qt{i}", tag=f"qt{i}")
               for i in range(NIB)]
        # K per block: [128, 4 chunks, 2, 128]; chunks duplicated for the
        # DoubleRow stationary pair
        ktb = [pp.tile([P, NDC, 2, P], fp8, name=f"kt{i}", tag=f"kt{i}")
               for i in range(NIB)]
        # V per block: [128 keys, 4 chunks, 2 heads, 66]; col 64 = ones
        # (66 not 65: even inner dim so the f32-bitcast memset works)
        vtb = [pp.tile([P, NDC, 2, DH + 2], fp16, name=f"vt{i}", tag=f"vt{i}")
               for i in range(NIB)]
        wq_sb = pp.tile([P, NDC, 2 * DH], fp16, name="wq_sb", tag="wq")
        wk_sb = pp.tile([P, NDC, 2 * DH], fp16, name="wk_sb", tag="wk")
        wv_sb = pp.tile([P, NDC, 2 * DH], fp16, name="wv_sb", tag="wv")
        wo_sb = pp.tile([DH + 1, 2, D], f32r, name="wo_sb", tag="wo")
        eb = pp.tile([P, 1], f32, name="eb", tag="eb")
        # normalized O operands (ones row at partition DH feeds the bias row)
        ots0 = pp.tile([DH + 1, IB], f32r, name="ots0", tag="ots0")
        ots1 = pp.tile([DH + 1, IB], f32r, name="ots1", tag="ots1")

        nc.sync.dma_start(wq_sb[:, :, :], wq[:, :, :])
        nc.sync.dma_start(wk_sb[:, :, :], wk[:, :, :])
        nc.sync.dma_start(wv_sb[:, :, :], wv[:, :, :])
        nc.sync.dma_start(xtb[0][:, :, :], xT[:, :, 0:IB])
        nc.sync.dma_start(wo_sb[:, :, :], wo[:, :])
        for b in range(1, NIB):
            nc.sync.dma_start(xtb[b][:, :, :],
                              xT[:, :, b * IB:(b + 1) * IB])
        nc.vector.memset(eb[:], float(np.exp(SCALE)))
        nc.vector.memset(ots0[DH:DH + 1, :].bitcast(f32), 1.0)
        nc.vector.memset(ots1[DH:DH + 1, :].bitcast(f32), 1.0)
        # two packed fp16 1.0 values as an f32 bit pattern
        ones_f16x2 = float(np.frombuffer(b"\x00\x3c\x00\x3c", np.float32)[0])
        for i in range(NIB):
            # whole vt = fp16 ones; V copies later overwrite cols 0:DH
            nc.vector.memset(vtb[i][:, :, :, :].bitcast(f32), ones_f16x2)

        with tc.tile_pool(name="ps", bufs=LA, space="PSUM") as ps_pool, \
             tc.tile_pool(name="po", bufs=1, space="PSUM") as po_pool, \
             tc.tile_pool(name="pt", bufs=8) as pt_pool, \
             tc.tile_pool(name="sc", bufs=4) as sc_pool, \
             tc.tile_pool(name="ep", bufs=2) as ep_pool, \
             tc.tile_pool(name="ot", bufs=3) as ot_pool:

            def phase_a(b):
                """Project block b's K, Q (-> fp8) and V (-> fp16)."""
                xt = xtb[b]
                pk = ps_pool.tile([P, NDC, P], f32, tag="st", name="pk")
                for dc in range(NDC):
                    nc.tensor.matmul(pk[:, :, :], wk_sb[:, dc, :], xt[:, dc, :],
                                     start=(dc == 0), stop=(dc == NDC - 1))
                nc.vector.tensor_copy(
                    ktb[b][:, :, :, :],
                    pk[:, :, :].unsqueeze(2).broadcast_to((P, NDC, 2, P)))
                pq = ps_pool.tile([P, IB], f32, tag="st", name="pq")
                for dc in range(NDC):
                    nc.tensor.matmul(pq[:], wq_sb[:, dc, :], xt[:, dc, :],
                                     start=(dc == 0), stop=(dc == NDC - 1))
                nc.vector.tensor_copy(qtb[b][:, 0, :], pq[:])
                # Q_lo = Q - fp8(Q), rounded to fp8 (subnormals cover it)
                with nc.allow_low_precision(reason="double-fp8 residual"):
                    nc.vector.tensor_tensor(qtb[b][:, 1, :], pq[:],
                                            qtb[b][:, 0, :], Sub)
                pv = ps_pool.tile([P, NDC, 2, DH], f32, tag="st", name="pv")
                for q in range(NDC):
                    for dc in range(NDC):
                        nc.tensor.matmul(
                            pv[:, q, :, :], xt[:, dc, q * P:(q + 1) * P],
                            wv_sb[:, dc, :],
                            start=(dc == 0), stop=(dc == NDC - 1))
                nc.vector.tensor_copy(vtb[b][:, :, :, 0:DH], pv[:, :, :, :])

            def emit_s(ib, jc):
                """S matmuls + exp for key-chunk jc of query block ib."""
                qt = qtb[ib]
                blk, c = jc // NDC, jc % NDC
                kt = ktb[blk]
                st = ps_pool.tile([P, 2 * IB], f32, tag="st", name="st")
                nc.tensor.matmul(st[:, 0:IB], kt[0:DH, c, :, :],
                                 qt[0:DH, :, :],
                                 start=True, stop=True, perf_mode=DR)
                nc.tensor.matmul(st[:, IB:2 * IB], kt[DH:P, c, :, :],
                                 qt[DH:P, :, :],
                                 start=True, stop=True, perf_mode=DR,
                                 tile_position=(64, 0))
                pt = pt_pool.tile([P, 2 * IB], fp16, tag="pt", name="pt")
                if is_pool(jc):
                    sc = sc_pool.tile([P, 2 * IB], fp16, tag="sc", name="sc")
                    nc.vector.tensor_copy(sc[:], st[:])
                    nc.gpsimd.tensor_tensor(
                        pt[:], eb[:].broadcast_to((P, 2 * IB)), sc[:], Pow)
                else:
                    nc.scalar.activation(pt[:], st[:], Exp, scale=SCALE)
                return pt

            def emit_o(o0, o1, jc, pt, first, last):
                blk, c = jc // NDC, jc % NDC
                vt = vtb[blk]
                nc.tensor.matmul(o0[:], vt[:, c, 0, 0:DH + 1], pt[:, 0:IB],
                                 start=first, stop=last)
                nc.tensor.matmul(o1[:], vt[:, c, 1, 0:DH + 1], pt[:, IB:2 * IB],
                                 start=first, stop=last)

            def epi_norm(o0, o1):
                """Drain O accumulators to SBUF (frees their PSUM bank) and
                compute the denominators' reciprocals."""
                oc0 = ep_pool.tile([DH + 1, IB], f32, tag="oc0", name="oc0")
                oc1 = ep_pool.tile([DH + 1, IB], f32, tag="oc1", name="oc1")
                nc.vector.tensor_copy(oc0[:], o0[:])
                nc.vector.tensor_copy(oc1[:], o1[:])
                rv0 = ep_pool.tile([1, IB], f32, tag="rv0", name="rv0")
                rv1 = ep_pool.tile([1, IB], f32, tag="rv1", name="rv1")
                nc.vector.reciprocal(rv0[:], oc0[DH:DH + 1, :])
                nc.vector.reciprocal(rv1[:], oc1[DH:DH + 1, :])
                return oc0, oc1, (rv0, rv1)

            def epi_scale(oc0, oc1, rv):
                """ots_h = O_h * (1/denom) broadcast per query."""
                rv0, rv1 = rv
                rb0 = ep_pool.tile([DH, IB], f32, tag="rb0", name="rb0")
                rb1 = ep_pool.tile([DH, IB], f32, tag="rb1", name="rb1")
                nc.gpsimd.partition_broadcast(rb0[:], rv0[:])
                nc.gpsimd.partition_broadcast(rb1[:], rv1[:])
                nc.vector.tensor_tensor(ots0[0:DH, :], oc0[0:DH, :],
                                        rb0[:], Mult)
                nc.vector.tensor_tensor(ots1[0:DH, :], oc1[0:DH, :],
                                        rb1[:], Mult)

            def epi_proj(ib, half):
                """Output projection of 2 query chunks (half in {0,1})."""
                ppx = ps_pool.tile([P, 2, D], f32, tag="st", name="ppx")
                for qi in range(2):
                    q = half * 2 + qi
                    q0, q1 = q * P, (q + 1) * P
                    nc.tensor.matmul(ppx[:, qi, :], ots0[:, q0:q1],
                                     wo_sb[:, 0, :], start=True, stop=False)
                    nc.tensor.matmul(ppx[:, qi, :], ots1[:, q0:q1],
                                     wo_sb[:, 1, :], start=False, stop=True)
                otile = ot_pool.tile([P, 2, D], f32, tag="ot", name="otile")
                nc.vector.tensor_copy(otile[:, :, :], ppx[:, :, :])
                r0 = ib * IB + half * 2 * P
                nc.sync.dma_start(out[r0:r0 + P, :], otile[:, 0, :])
                nc.sync.dma_start(out[r0 + P:r0 + 2 * P, :], otile[:, 1, :])

            # O-emission order: pool-path chunks drop POOL_LAG positions
            # later so the PE never waits on their longer exp chain
            o_order = sorted(range(NJC),
                             key=lambda c: (c + (POOL_LAG if is_pool(c) else 0),
                                            c))
            pend = None  # (ib, oc0, oc1, rv) awaiting scale+projection
            for ib in range(NIB):
                if ib == 0:
                    phase_a(0)
                    phase_a(1)
                o0 = po_pool.tile([DH + 1, IB], f32, tag="o0", name="o0")
                o1 = po_pool.tile([DH + 1, IB], f32, tag="o1", name="o1")
                pts = {}
                for j in range(LA):
                    pts[j] = emit_s(ib, j)
                for k in range(NJC):
                    if k + LA < NJC:
                        pts[k + LA] = emit_s(ib, k + LA)
                    c = o_order[k]
                    emit_o(o0, o1, c, pts.pop(c),
                           first=(k == 0), last=(k == NJC - 1))
                    if ib == 0:
                        if k >= 2 and (k - 2) % 4 == 0 and (k - 2) // 4 + 2 < NIB:
                            phase_a((k - 2) // 4 + 2)
                    elif pend is not None:
                        if k == 2:
                            epi_scale(pend[1], pend[2], pend[3])
                        elif k == 5:
                            epi_proj(pend[0], 0)
                        elif k == 8:
                            epi_proj(pend[0], 1)
                            pend = None
                oc0, oc1, rv = epi_norm(o0, o1)
                pend = (ib, oc0, oc1, rv)
            epi_scale(pend[1], pend[2], pend[3])
            epi_proj(pend[0], 0)
            epi_proj(pend[0], 1)

    nc.compile()
    return nc


def _get_nc():
    if "nc" not in _CACHE:
        _CACHE["nc"] = _build()
    return _CACHE["nc"]


def kernel(x, Wq, Wk, Wv, Wo, bo):
    from concourse.bass_utils import run_bass_kernel_spmd

    x = np.asarray(x, dtype=np.float32)
    Wq = np.asarray(Wq, dtype=np.float32)
    Wk = np.asarray(Wk, dtype=np.float32)
    Wv = np.asarray(Wv, dtype=np.float32)
    Wo = np.asarray(Wo, dtype=np.float32)
    bo = np.asarray(bo, dtype=np.float32)

    nc = _get_nc()

    def prechunk(a):  # [512, M] -> [128, 4, M] with row r = dc*128+p
        return np.ascontiguousarray(
            a.reshape(NDC, P, a.shape[1]).transpose(1, 0, 2)).astype(np.float16)

    xTs = [prechunk(x[b].T) for b in range(B)]
    in_maps = []
    for c in range(8):
        b, p = c // 4, c % 4
        sl = slice(p * 2 * DH, (p + 1) * 2 * DH)
        wo_aug = np.zeros((DH + 1, 2 * D), dtype=np.float32)
        wo_aug[0:DH, 0:D] = Wo[sl, :][0:DH, :]
        wo_aug[0:DH, D:2 * D] = Wo[sl, :][DH:2 * DH, :]
        if p == 0:
            wo_aug[DH, 0:D] = bo
        in_maps.append({
            "xT": xTs[b],
            "wq": prechunk(Wq[:, sl]),
            "wk": prechunk(Wk[:, sl]),
            "wv": prechunk(Wv[:, sl]),
            "wo": wo_aug,
        })

    try:
        res = run_bass_kernel_spmd(nc, in_maps, core_ids=list(range(8)))
    except Exception:
        # transient device wedge (NRT_EXEC_UNIT_UNRECOVERABLE) — retry once
        import time as _time
        _time.sleep(45)
        res = run_bass_kernel_spmd(nc, in_maps, core_ids=list(range(8)))
    parts = [res.results[c]["out"] for c in range(8)]
    full = np.stack([
        parts[0] + parts[1] + parts[2] + parts[3],
        parts[4] + parts[5] + parts[6] + parts[7],
    ]).astype(np.float32)
    return full


# revision 3
# speedup vs baseline: 1.0162x; 1.0039x over previous
"""Bass/Tile TRN2 kernel v5 for CrossAttention (B=2, N=4096, D=512, H=8, DH=64).

Sharding: batch*heads over 8 cores — core c handles batch c//4 and heads
(c%4)*2, (c%4)*2+1.

Design (cost-model driven):
  - S = K^T Q in fp8e4 DoubleRow (0.5 cycles/row). The two DoubleRow k-tiles
    carry (Q_hi, Q_lo) — a double-fp8 split of Q — against duplicated fp8 K,
    so only K's fp8 rounding reaches the logits (~1e-2 Frobenius).
  - exp split across engines: 20/32 key-chunks on ACT (exp(S*SCALE)), 12/32
    via GPSIMD pow (DVE stages S to SBUF fp16, Pool computes (e^SCALE)**S).
  - O = [V|1]^T P in fp16; denominator rides as V's 65th column.
  - Projections (phase A) interleave with the first query block, sharing the
    S-tile PSUM slots; S pipeline runs with 3-chunk lookahead.
  - Per-block epilogue is split so the PE never starves: at the block
    boundary only two DVE copies drain the O accumulators (freeing their
    PSUM bank for the next block); the normalization (reciprocal, a PE
    broadcast-matmul, DVE multiplies) and the output projection run spread
    over the next block.
"""

import sys

if "/opt/trn_rl_repo" not in sys.path:
    sys.path.insert(0, "/opt/trn_rl_repo")

import numpy as np

B, N, D = 2, 4096, 512
H, DH = 8, 64
SCALE = DH ** -0.5
P = 128
IB = 512            # i/query block
NDC = D // P        # 4 contraction chunks for projections
NIB = N // IB       # 8
NJC = N // P        # 32 key chunks per query block
LA = 3              # S-matmul chunks in flight (= st pool bufs)

# key-chunk indices handled by the GPSIMD pow path (rest go to ACT exp)
def is_pool(jc):
    return jc % 3 == 1  # 11 of 32


def pool_lag(jc):
    return POOL_LAG
# extra O-emission lag for pool-path chunks (their exp chain is longer)
POOL_LAG = 4

_CACHE: dict = {}


def _build():
    import concourse.mybir as mybir
    from concourse import bacc
    from concourse.tile import TileContext

    f32 = mybir.dt.float32
    f32r = mybir.dt.float32r
    fp16 = mybir.dt.float16
    fp8 = mybir.dt.float8e4
    Exp = mybir.ActivationFunctionType.Exp
    DR = mybir.MatmulPerfMode.DoubleRow
    Pow = mybir.AluOpType.pow
    Mult = mybir.AluOpType.mult
    Sub = mybir.AluOpType.subtract

    nc = bacc.Bacc("TRN2")
    # host pre-chunks contraction-dim tiles: [128, NDC, ...]
    xT = nc.dram_tensor("xT", [P, NDC, N], fp16, kind="ExternalInput")
    wq = nc.dram_tensor("wq", [P, NDC, 2 * DH], fp16, kind="ExternalInput")
    wk = nc.dram_tensor("wk", [P, NDC, 2 * DH], fp16, kind="ExternalInput")
    wv = nc.dram_tensor("wv", [P, NDC, 2 * DH], fp16, kind="ExternalInput")
    # augmented output projection: [Wo_h0; bo | Wo_h1; 0], shape [DH+1, 2*D]
    wo = nc.dram_tensor("wo", [DH + 1, 2 * D], f32r, kind="ExternalInput")
    out = nc.dram_tensor("out", [N, D], f32, kind="ExternalOutput")

    with TileContext(nc) as tc, \
         tc.tile_pool(name="persist", bufs=1) as pp:
        xtb = [pp.tile([P, NDC, IB], fp16, name=f"xt{i}", tag=f"xt{i}")
               for i in range(NIB)]
        # Q per block: [128 (2 heads x 64 d), 2 k-tiles, 512] = (Q_hi, Q_lo)
        qtb = [pp.tile([P, 2, IB], fp8, name=f"qt{i}", tag=f"qt{i}")
               for i in range(NIB)]
        # K per block: [128, 4 chunks, 2, 128]; chunks duplicated for the
        # DoubleRow stationary pair
        ktb = [pp.tile([P, NDC, 2, P], fp8, name=f"kt{i}", tag=f"kt{i}")
               for i in range(NIB)]
        # V per block: [128 keys, 4 chunks, 2 heads, 66]; col 64 = ones
        # (66 not 65: even inner dim so the f32-bitcast memset works)
        vtb = [pp.tile([P, NDC, 2, DH + 2], fp16, name=f"vt{i}", tag=f"vt{i}")
               for i in range(NIB)]
        wq_sb = pp.tile([P, NDC, 2 * DH], fp16, name="wq_sb", tag="wq")
        wk_sb = pp.tile([P, NDC, 2 * DH], fp16, name="wk_sb", tag="wk")
        wv_sb = pp.tile([P, NDC, 2 * DH], fp16, name="wv_sb", tag="wv")
        wo_sb = pp.tile([DH + 1, 2, D], f32r, name="wo_sb", tag="wo")
        eb = pp.tile([P, 1], f32, name="eb", tag="eb")
        # normalized O operands (ones row at partition DH feeds the bias row)
        ots0 = pp.tile([DH + 1, IB], f32r, name="ots0", tag="ots0")
        ots1 = pp.tile([DH + 1, IB], f32r, name="ots1", tag="ots1")

        nc.sync.dma_start(wq_sb[:, :, :], wq[:, :, :])
        nc.sync.dma_start(wk_sb[:, :, :], wk[:, :, :])
        nc.sync.dma_start(wv_sb[:, :, :], wv[:, :, :])
        nc.sync.dma_start(xtb[0][:, :, :], xT[:, :, 0:IB])
        nc.sync.dma_start(wo_sb[:, :, :], wo[:, :])
        for b in range(1, NIB):
            nc.sync.dma_start(xtb[b][:, :, :],
                              xT[:, :, b * IB:(b + 1) * IB])
        nc.vector.memset(eb[:], float(np.exp(SCALE)))
        nc.vector.memset(ots0[DH:DH + 1, :].bitcast(f32), 1.0)
        nc.vector.memset(ots1[DH:DH + 1, :].bitcast(f32), 1.0)
        # two packed fp16 1.0 values as an f32 bit pattern
        ones_f16x2 = float(np.frombuffer(b"\x00\x3c\x00\x3c", np.float32)[0])
        for i in range(NIB):
            # whole vt = fp16 ones; V copies later overwrite cols 0:DH
            nc.vector.memset(vtb[i][:, :, :, :].bitcast(f32), ones_f16x2)

        with tc.tile_pool(name="ps", bufs=LA, space="PSUM") as ps_pool, \
             tc.tile_pool(name="po", bufs=1, space="PSUM") as po_pool, \
             tc.tile_pool(name="pt", bufs=10) as pt_pool, \
             tc.tile_pool(name="sc", bufs=6) as sc_pool, \
             tc.tile_pool(name="ep", bufs=2) as ep_pool, \
             tc.tile_pool(name="ot", bufs=3) as ot_pool:

            def phase_a(b):
                """Project block b's K, Q (-> fp8) and V (-> fp16)."""
                xt = xtb[b]
                pk = ps_pool.tile([P, NDC, P], f32, tag="st", name="pk")
                for dc in range(NDC):
                    nc.tensor.matmul(pk[:, :, :], wk_sb[:, dc, :], xt[:, dc, :],
                                     start=(dc == 0), stop=(dc == NDC - 1))
                nc.vector.tensor_copy(
                    ktb[b][:, :, :, :],
                    pk[:, :, :].unsqueeze(2).broadcast_to((P, NDC, 2, P)))
                pq = ps_pool.tile([P, IB], f32, tag="st", name="pq")
                for dc in range(NDC):
                    nc.tensor.matmul(pq[:], wq_sb[:, dc, :], xt[:, dc, :],
                                     start=(dc == 0), stop=(dc == NDC - 1))
                nc.vector.tensor_copy(qtb[b][:, 0, :], pq[:])
                # Q_lo = Q - fp8(Q), rounded to fp8 (subnormals cover it)
                with nc.allow_low_precision(reason="double-fp8 residual"):
                    nc.vector.tensor_tensor(qtb[b][:, 1, :], pq[:],
                                            qtb[b][:, 0, :], Sub)
                pv = ps_pool.tile([P, NDC, 2, DH], f32, tag="st", name="pv")
                for q in range(NDC):
                    for dc in range(NDC):
                        nc.tensor.matmul(
                            pv[:, q, :, :], xt[:, dc, q * P:(q + 1) * P],
                            wv_sb[:, dc, :],
                            start=(dc == 0), stop=(dc == NDC - 1))
                nc.vector.tensor_copy(vtb[b][:, :, :, 0:DH], pv[:, :, :, :])

            def emit_s(ib, jc):
                """S matmuls + exp for key-chunk jc of query block ib."""
                qt = qtb[ib]
                blk, c = jc // NDC, jc % NDC
                kt = ktb[blk]
                st = ps_pool.tile([P, 2 * IB], f32, tag="st", name="st")
                nc.tensor.matmul(st[:, 0:IB], kt[0:DH, c, :, :],
                                 qt[0:DH, :, :],
                                 start=True, stop=True, perf_mode=DR)
                nc.tensor.matmul(st[:, IB:2 * IB], kt[DH:P, c, :, :],
                                 qt[DH:P, :, :],
                                 start=True, stop=True, perf_mode=DR,
                                 tile_position=(64, 0))
                pt = pt_pool.tile([P, 2 * IB], fp16, tag="pt", name="pt")
                if is_pool(jc):
                    sc = sc_pool.tile([P, 2 * IB], fp16, tag="sc", name="sc")
                    nc.vector.tensor_copy(sc[:], st[:])
                    nc.gpsimd.tensor_tensor(
                        pt[:], eb[:].broadcast_to((P, 2 * IB)), sc[:], Pow)
                else:
                    nc.scalar.activation(pt[:], st[:], Exp, scale=SCALE)
                return pt

            def emit_o(o0, o1, jc, pt, first, last):
                blk, c = jc // NDC, jc % NDC
                vt = vtb[blk]
                nc.tensor.matmul(o0[:], vt[:, c, 0, 0:DH + 1], pt[:, 0:IB],
                                 start=first, stop=last)
                nc.tensor.matmul(o1[:], vt[:, c, 1, 0:DH + 1], pt[:, IB:2 * IB],
                                 start=first, stop=last)

            def epi_norm(o0, o1):
                """Drain O accumulators to SBUF (frees their PSUM bank) and
                compute the denominators' reciprocals."""
                oc0 = ep_pool.tile([DH + 1, IB], f32, tag="oc0", name="oc0")
                oc1 = ep_pool.tile([DH + 1, IB], f32, tag="oc1", name="oc1")
                nc.vector.tensor_copy(oc0[:], o0[:])
                nc.vector.tensor_copy(oc1[:], o1[:])
                rv0 = ep_pool.tile([1, IB], f32, tag="rv0", name="rv0")
                rv1 = ep_pool.tile([1, IB], f32, tag="rv1", name="rv1")
                nc.vector.reciprocal(rv0[:], oc0[DH:DH + 1, :])
                nc.vector.reciprocal(rv1[:], oc1[DH:DH + 1, :])
                return oc0, oc1, (rv0, rv1)

            def epi_scale(oc0, oc1, rv):
                """ots_h = O_h * (1/denom) broadcast per query."""
                rv0, rv1 = rv
                rb0 = ep_pool.tile([DH, IB], f32, tag="rb0", name="rb0")
                rb1 = ep_pool.tile([DH, IB], f32, tag="rb1", name="rb1")
                nc.gpsimd.partition_broadcast(rb0[:], rv0[:])
                nc.gpsimd.partition_broadcast(rb1[:], rv1[:])
                nc.vector.tensor_tensor(ots0[0:DH, :], oc0[0:DH, :],
                                        rb0[:], Mult)
                nc.vector.tensor_tensor(ots1[0:DH, :], oc1[0:DH, :],
                                        rb1[:], Mult)

            def epi_proj(ib, half):
                """Output projection of 2 query chunks (half in {0,1})."""
                ppx = ps_pool.tile([P, 2, D], f32, tag="st", name="ppx")
                for qi in range(2):
                    q = half * 2 + qi
                    q0, q1 = q * P, (q + 1) * P
                    nc.tensor.matmul(ppx[:, qi, :], ots0[:, q0:q1],
                                     wo_sb[:, 0, :], start=True, stop=False)
                    nc.tensor.matmul(ppx[:, qi, :], ots1[:, q0:q1],
                                     wo_sb[:, 1, :], start=False, stop=True)
                otile = ot_pool.tile([P, 2, D], f32, tag="ot", name="otile")
                nc.vector.tensor_copy(otile[:, :, :], ppx[:, :, :])
                r0 = ib * IB + half * 2 * P
                nc.sync.dma_start(out[r0:r0 + P, :], otile[:, 0, :])
                nc.sync.dma_start(out[r0 + P:r0 + 2 * P, :], otile[:, 1, :])

            # O-emission order: pool-path chunks drop POOL_LAG positions
            # later so the PE never waits on their longer exp chain
            o_order = sorted(range(NJC),
                             key=lambda c: (c + (pool_lag(c) if is_pool(c)
                                                 else 0), c))
            phase_a(0)
            phase_a(1)
            pend = None  # (ib, oc0, oc1, rv) awaiting scale+projection
            pts = {}
            o0 = o1 = None
            NG = NIB * NJC
            for j in range(LA):
                pts[j] = emit_s(j // NJC, j % NJC)
            for g in range(NG):
                ib, k = g // NJC, g % NJC
                if k == 0:
                    if ib > 0:
                        oc0, oc1, rv = epi_norm(o0, o1)
                        pend = (ib - 1, oc0, oc1, rv)
                    o0 = po_pool.tile([DH + 1, IB], f32, tag="o0", name="o0")
                    o1 = po_pool.tile([DH + 1, IB], f32, tag="o1", name="o1")
                if g + LA < NG:
                    gs = g + LA
                    pts[gs] = emit_s(gs // NJC, gs % NJC)
                c = o_order[k]
                emit_o(o0, o1, c, pts.pop(ib * NJC + c),
                       first=(k == 0), last=(k == NJC - 1))
                if ib == 0:
                    if k >= 2 and (k - 2) % 4 == 0 and (k - 2) // 4 + 2 < NIB:
                        phase_a((k - 2) // 4 + 2)
                elif pend is not None:
                    if k == 2:
                        epi_scale(pend[1], pend[2], pend[3])
                    elif k == 5:
                        epi_proj(pend[0], 0)
                    elif k == 8:
                        epi_proj(pend[0], 1)
                        pend = None
            oc0, oc1, rv = epi_norm(o0, o1)
            epi_scale(oc0, oc1, rv)
            epi_proj(NIB - 1, 0)
            epi_proj(NIB - 1, 1)

    nc.compile()
    return nc


def _get_nc():
    if "nc" not in _CACHE:
        _CACHE["nc"] = _build()
    return _CACHE["nc"]


def kernel(x, Wq, Wk, Wv, Wo, bo):
    from concourse.bass_utils import run_bass_kernel_spmd

    x = np.asarray(x, dtype=np.float32)
    Wq = np.asarray(Wq, dtype=np.float32)
    Wk = np.asarray(Wk, dtype=np.float32)
    Wv = np.asarray(Wv, dtype=np.float32)
    Wo = np.asarray(Wo, dtype=np.float32)
    bo = np.asarray(bo, dtype=np.float32)

    nc = _get_nc()

    def prechunk(a):  # [512, M] -> [128, 4, M] with row r = dc*128+p
        return np.ascontiguousarray(
            a.reshape(NDC, P, a.shape[1]).transpose(1, 0, 2)).astype(np.float16)

    xTs = [prechunk(x[b].T) for b in range(B)]
    in_maps = []
    for c in range(8):
        b, p = c // 4, c % 4
        sl = slice(p * 2 * DH, (p + 1) * 2 * DH)
        wo_aug = np.zeros((DH + 1, 2 * D), dtype=np.float32)
        wo_aug[0:DH, 0:D] = Wo[sl, :][0:DH, :]
        wo_aug[0:DH, D:2 * D] = Wo[sl, :][DH:2 * DH, :]
        if p == 0:
            wo_aug[DH, 0:D] = bo
        in_maps.append({
            "xT": xTs[b],
            "wq": prechunk(Wq[:, sl]),
            "wk": prechunk(Wk[:, sl]),
            "wv": prechunk(Wv[:, sl]),
            "wo": wo_aug,
        })

    try:
        res = run_bass_kernel_spmd(nc, in_maps, core_ids=list(range(8)))
    except Exception:
        # transient device wedge (NRT_EXEC_UNIT_UNRECOVERABLE) — retry once
        import time as _time
        _time.sleep(45)
        res = run_bass_kernel_spmd(nc, in_maps, core_ids=list(range(8)))
    parts = [res.results[c]["out"] for c in range(8)]
    full = np.stack([
        parts[0] + parts[1] + parts[2] + parts[3],
        parts[4] + parts[5] + parts[6] + parts[7],
    ]).astype(np.float32)
    return full


# revision 4
# speedup vs baseline: 1.0188x; 1.0025x over previous
"""Bass/Tile TRN2 kernel v5 for CrossAttention (B=2, N=4096, D=512, H=8, DH=64).

Sharding: batch*heads over 8 cores — core c handles batch c//4 and heads
(c%4)*2, (c%4)*2+1.

Design (cost-model driven):
  - S = K^T Q in fp8e4 DoubleRow (0.5 cycles/row). The two DoubleRow k-tiles
    carry (Q_hi, Q_lo) — a double-fp8 split of Q — against duplicated fp8 K,
    so only K's fp8 rounding reaches the logits (~1e-2 Frobenius).
  - exp split across engines: 20/32 key-chunks on ACT (exp(S*SCALE)), 12/32
    via GPSIMD pow (DVE stages S to SBUF fp16, Pool computes (e^SCALE)**S).
  - O = [V|1]^T P in fp16; denominator rides as V's 65th column.
  - Projections (phase A) interleave with the first query block, sharing the
    S-tile PSUM slots; S pipeline runs with 3-chunk lookahead.
  - Per-block epilogue is split so the PE never starves: at the block
    boundary only two DVE copies drain the O accumulators (freeing their
    PSUM bank for the next block); the normalization (reciprocal, a PE
    broadcast-matmul, DVE multiplies) and the output projection run spread
    over the next block.
"""

import sys

if "/opt/trn_rl_repo" not in sys.path:
    sys.path.insert(0, "/opt/trn_rl_repo")

import numpy as np

B, N, D = 2, 4096, 512
H, DH = 8, 64
SCALE = DH ** -0.5
P = 128
IB = 512            # i/query block
NDC = D // P        # 4 contraction chunks for projections
NIB = N // IB       # 8
NJC = N // P        # 32 key chunks per query block
LA = 3              # S-matmul chunks in flight (= st pool bufs)

# key-chunk indices handled by the GPSIMD pow path (rest go to ACT exp)
def is_pool(jc):
    return jc % 3 == 1  # 11 of 32


def pool_lag(jc):
    return POOL_LAG
# extra O-emission lag for pool-path chunks (their exp chain is longer)
POOL_LAG = 4

_CACHE: dict = {}


def _build():
    import concourse.mybir as mybir
    from concourse import bacc
    from concourse.tile import TileContext

    f32 = mybir.dt.float32
    f32r = mybir.dt.float32r
    fp16 = mybir.dt.float16
    fp8 = mybir.dt.float8e4
    Exp = mybir.ActivationFunctionType.Exp
    DR = mybir.MatmulPerfMode.DoubleRow
    Pow = mybir.AluOpType.pow
    Mult = mybir.AluOpType.mult
    Sub = mybir.AluOpType.subtract

    nc = bacc.Bacc("TRN2")
    # host pre-chunks contraction-dim tiles: [128, NDC, ...]
    xT = nc.dram_tensor("xT", [P, NDC, N], fp16, kind="ExternalInput")
    wq = nc.dram_tensor("wq", [P, NDC, 2 * DH], fp16, kind="ExternalInput")
    wk = nc.dram_tensor("wk", [P, NDC, 2 * DH], fp16, kind="ExternalInput")
    wv = nc.dram_tensor("wv", [P, NDC, 2 * DH], fp16, kind="ExternalInput")
    # augmented output projection: [Wo_h0; bo | Wo_h1; 0], shape [DH+1, 2*D]
    wo = nc.dram_tensor("wo", [DH + 1, 2 * D], f32r, kind="ExternalInput")
    out = nc.dram_tensor("out", [N, D], f32, kind="ExternalOutput")

    with TileContext(nc) as tc, \
         tc.tile_pool(name="persist", bufs=1) as pp:
        xtb = [pp.tile([P, NDC, IB], fp16, name=f"xt{i}", tag=f"xt{i}")
               for i in range(NIB)]
        # Q per block: [128 (2 heads x 64 d), 2 k-tiles, 512] = (Q_hi, Q_lo)
        qtb = [pp.tile([P, 2, IB], fp8, name=f"qt{i}", tag=f"qt{i}")
               for i in range(NIB)]
        # K per block: [128, 4 chunks, 2, 128]; chunks duplicated for the
        # DoubleRow stationary pair
        ktb = [pp.tile([P, NDC, 2, P], fp8, name=f"kt{i}", tag=f"kt{i}")
               for i in range(NIB)]
        # V per block: [128 keys, 4 chunks, 2 heads, 66]; col 64 = ones
        # (66 not 65: even inner dim so the f32-bitcast memset works)
        vtb = [pp.tile([P, NDC, 2, DH + 2], fp16, name=f"vt{i}", tag=f"vt{i}")
               for i in range(NIB)]
        wq_sb = pp.tile([P, NDC, 2 * DH], fp16, name="wq_sb", tag="wq")
        wk_sb = pp.tile([P, NDC, 2 * DH], fp16, name="wk_sb", tag="wk")
        wv_sb = pp.tile([P, NDC, 2 * DH], fp16, name="wv_sb", tag="wv")
        wo_sb = pp.tile([DH + 1, 2, D], f32r, name="wo_sb", tag="wo")
        eb = pp.tile([P, 1], f32, name="eb", tag="eb")
        # normalized O operands (ones row at partition DH feeds the bias row)
        ots0 = pp.tile([DH + 1, IB], f32r, name="ots0", tag="ots0")
        ots1 = pp.tile([DH + 1, IB], f32r, name="ots1", tag="ots1")

        nc.sync.dma_start(wq_sb[:, :, :], wq[:, :, :])
        nc.sync.dma_start(wk_sb[:, :, :], wk[:, :, :])
        nc.sync.dma_start(wv_sb[:, :, :], wv[:, :, :])
        nc.sync.dma_start(xtb[0][:, :, :], xT[:, :, 0:IB])
        nc.sync.dma_start(wo_sb[:, :, :], wo[:, :])
        for b in range(1, NIB):
            nc.sync.dma_start(xtb[b][:, :, :],
                              xT[:, :, b * IB:(b + 1) * IB])
        nc.vector.memset(eb[:], float(np.exp(SCALE)))
        nc.vector.memset(ots0[DH:DH + 1, :].bitcast(f32), 1.0)
        nc.vector.memset(ots1[DH:DH + 1, :].bitcast(f32), 1.0)
        # two packed fp16 1.0 values as an f32 bit pattern
        ones_f16x2 = float(np.frombuffer(b"\x00\x3c\x00\x3c", np.float32)[0])
        for i in range(NIB):
            # whole vt = fp16 ones; V copies later overwrite cols 0:DH
            nc.vector.memset(vtb[i][:, :, :, :].bitcast(f32), ones_f16x2)

        with tc.tile_pool(name="ps", bufs=LA, space="PSUM") as ps_pool, \
             tc.tile_pool(name="po", bufs=1, space="PSUM") as po_pool, \
             tc.tile_pool(name="pt", bufs=10) as pt_pool, \
             tc.tile_pool(name="sc", bufs=6) as sc_pool, \
             tc.tile_pool(name="ep", bufs=2) as ep_pool, \
             tc.tile_pool(name="ot", bufs=3) as ot_pool:

            def phase_a(b):
                """Project block b's K, Q (-> fp8) and V (-> fp16)."""
                xt = xtb[b]
                pk = ps_pool.tile([P, NDC, P], f32, tag="st", name="pk")
                for dc in range(NDC):
                    nc.tensor.matmul(pk[:, :, :], wk_sb[:, dc, :], xt[:, dc, :],
                                     start=(dc == 0), stop=(dc == NDC - 1))
                nc.vector.tensor_copy(
                    ktb[b][:, :, :, :],
                    pk[:, :, :].unsqueeze(2).broadcast_to((P, NDC, 2, P)))
                pq = ps_pool.tile([P, IB], f32, tag="st", name="pq")
                for dc in range(NDC):
                    nc.tensor.matmul(pq[:], wq_sb[:, dc, :], xt[:, dc, :],
                                     start=(dc == 0), stop=(dc == NDC - 1))
                nc.vector.tensor_copy(qtb[b][:, 0, :], pq[:])
                # Q_lo = Q - fp8(Q), rounded to fp8 (subnormals cover it)
                with nc.allow_low_precision(reason="double-fp8 residual"):
                    nc.vector.tensor_tensor(qtb[b][:, 1, :], pq[:],
                                            qtb[b][:, 0, :], Sub)
                pv = ps_pool.tile([P, NDC, 2, DH], f32, tag="st", name="pv")
                for q in range(NDC):
                    for dc in range(NDC):
                        nc.tensor.matmul(
                            pv[:, q, :, :], xt[:, dc, q * P:(q + 1) * P],
                            wv_sb[:, dc, :],
                            start=(dc == 0), stop=(dc == NDC - 1))
                nc.vector.tensor_copy(vtb[b][:, :, :, 0:DH], pv[:, :, :, :])

            def emit_s(ib, jc):
                """S matmuls + exp for key-chunk jc of query block ib."""
                qt = qtb[ib]
                blk, c = jc // NDC, jc % NDC
                kt = ktb[blk]
                st = ps_pool.tile([P, 2 * IB], f32, tag="st", name="st")
                nc.tensor.matmul(st[:, 0:IB], kt[0:DH, c, :, :],
                                 qt[0:DH, :, :],
                                 start=True, stop=True, perf_mode=DR)
                nc.tensor.matmul(st[:, IB:2 * IB], kt[DH:P, c, :, :],
                                 qt[DH:P, :, :],
                                 start=True, stop=True, perf_mode=DR,
                                 tile_position=(64, 0))
                pt = pt_pool.tile([P, 2 * IB], fp16, tag="pt", name="pt")
                if is_pool(jc):
                    sc = sc_pool.tile([P, 2 * IB], fp16, tag="sc", name="sc")
                    nc.vector.tensor_copy(sc[:], st[:])
                    nc.gpsimd.tensor_tensor(
                        pt[:], eb[:].broadcast_to((P, 2 * IB)), sc[:], Pow)
                else:
                    nc.scalar.activation(pt[:], st[:], Exp, scale=SCALE)
                return pt

            def emit_o(o0, o1, jc, pt, first, last):
                blk, c = jc // NDC, jc % NDC
                vt = vtb[blk]
                nc.tensor.matmul(o0[:], vt[:, c, 0, 0:DH + 1], pt[:, 0:IB],
                                 start=first, stop=last)
                nc.tensor.matmul(o1[:], vt[:, c, 1, 0:DH + 1], pt[:, IB:2 * IB],
                                 start=first, stop=last)

            def epi_norm(o0, o1):
                """Drain O accumulators to SBUF (frees their PSUM bank) and
                compute the denominators' reciprocals."""
                oc0 = ep_pool.tile([DH + 1, IB], f32, tag="oc0", name="oc0")
                oc1 = ep_pool.tile([DH + 1, IB], f32, tag="oc1", name="oc1")
                nc.vector.tensor_copy(oc0[:], o0[:])
                nc.vector.tensor_copy(oc1[:], o1[:])
                rv0 = ep_pool.tile([1, IB], f32, tag="rv0", name="rv0")
                rv1 = ep_pool.tile([1, IB], f32, tag="rv1", name="rv1")
                nc.vector.reciprocal(rv0[:], oc0[DH:DH + 1, :])
                nc.vector.reciprocal(rv1[:], oc1[DH:DH + 1, :])
                return oc0, oc1, (rv0, rv1)

            def epi_scale(oc0, oc1, rv):
                """ots_h = O_h * (1/denom) broadcast per query."""
                rv0, rv1 = rv
                rb0 = ep_pool.tile([DH, IB], f32, tag="rb0", name="rb0")
                rb1 = ep_pool.tile([DH, IB], f32, tag="rb1", name="rb1")
                nc.gpsimd.partition_broadcast(rb0[:], rv0[:])
                nc.gpsimd.partition_broadcast(rb1[:], rv1[:])
                nc.vector.tensor_tensor(ots0[0:DH, :], oc0[0:DH, :],
                                        rb0[:], Mult)
                nc.vector.tensor_tensor(ots1[0:DH, :], oc1[0:DH, :],
                                        rb1[:], Mult)

            def epi_proj(ib, q):
                """Output projection of one 128-query chunk."""
                ppx = ps_pool.tile([P, D], f32, tag="st", name="ppx")
                q0, q1 = q * P, (q + 1) * P
                nc.tensor.matmul(ppx[:, :], ots0[:, q0:q1],
                                 wo_sb[:, 0, :], start=True, stop=False)
                nc.tensor.matmul(ppx[:, :], ots1[:, q0:q1],
                                 wo_sb[:, 1, :], start=False, stop=True)
                otile = ot_pool.tile([P, D], f32, tag="ot", name="otile")
                nc.vector.tensor_copy(otile[:, :], ppx[:, :])
                r0 = ib * IB + q * P
                nc.sync.dma_start(out[r0:r0 + P, :], otile[:, :])

            # O-emission order: pool-path chunks drop POOL_LAG positions
            # later so the PE never waits on their longer exp chain
            o_order = sorted(range(NJC),
                             key=lambda c: (c + (pool_lag(c) if is_pool(c)
                                                 else 0), c))
            phase_a(0)
            phase_a(1)
            pend = None  # (ib, oc0, oc1, rv) awaiting scale+projection
            pts = {}
            o0 = o1 = None
            NG = NIB * NJC
            for j in range(LA):
                pts[j] = emit_s(j // NJC, j % NJC)
            for g in range(NG):
                ib, k = g // NJC, g % NJC
                if k == 0:
                    if ib > 0:
                        oc0, oc1, rv = epi_norm(o0, o1)
                        pend = (ib - 1, oc0, oc1, rv)
                    o0 = po_pool.tile([DH + 1, IB], f32, tag="o0", name="o0")
                    o1 = po_pool.tile([DH + 1, IB], f32, tag="o1", name="o1")
                if g + LA < NG:
                    gs = g + LA
                    pts[gs] = emit_s(gs // NJC, gs % NJC)
                c = o_order[k]
                emit_o(o0, o1, c, pts.pop(ib * NJC + c),
                       first=(k == 0), last=(k == NJC - 1))
                if ib == 0:
                    if k >= 2 and (k - 2) % 4 == 0 and (k - 2) // 4 + 2 < NIB:
                        phase_a((k - 2) // 4 + 2)
                elif pend is not None:
                    if k == 2:
                        epi_scale(pend[1], pend[2], pend[3])
                    elif k in (5, 7, 9, 11):
                        epi_proj(pend[0], (k - 5) // 2)
                        if k == 11:
                            pend = None
            oc0, oc1, rv = epi_norm(o0, o1)
            epi_scale(oc0, oc1, rv)
            for q in range(NDC):
                epi_proj(NIB - 1, q)

    nc.compile()
    return nc


def _get_nc():
    if "nc" not in _CACHE:
        _CACHE["nc"] = _build()
    return _CACHE["nc"]


def kernel(x, Wq, Wk, Wv, Wo, bo):
    from concourse.bass_utils import run_bass_kernel_spmd

    x = np.asarray(x, dtype=np.float32)
    Wq = np.asarray(Wq, dtype=np.float32)
    Wk = np.asarray(Wk, dtype=np.float32)
    Wv = np.asarray(Wv, dtype=np.float32)
    Wo = np.asarray(Wo, dtype=np.float32)
    bo = np.asarray(bo, dtype=np.float32)

    nc = _get_nc()

    def prechunk(a):  # [512, M] -> [128, 4, M] with row r = dc*128+p
        return np.ascontiguousarray(
            a.reshape(NDC, P, a.shape[1]).transpose(1, 0, 2)).astype(np.float16)

    xTs = [prechunk(x[b].T) for b in range(B)]
    in_maps = []
    for c in range(8):
        b, p = c // 4, c % 4
        sl = slice(p * 2 * DH, (p + 1) * 2 * DH)
        wo_aug = np.zeros((DH + 1, 2 * D), dtype=np.float32)
        wo_aug[0:DH, 0:D] = Wo[sl, :][0:DH, :]
        wo_aug[0:DH, D:2 * D] = Wo[sl, :][DH:2 * DH, :]
        if p == 0:
            wo_aug[DH, 0:D] = bo
        in_maps.append({
            "xT": xTs[b],
            "wq": prechunk(Wq[:, sl]),
            "wk": prechunk(Wk[:, sl]),
            "wv": prechunk(Wv[:, sl]),
            "wo": wo_aug,
        })

    try:
        res = run_bass_kernel_spmd(nc, in_maps, core_ids=list(range(8)))
    except Exception:
        # transient device wedge (NRT_EXEC_UNIT_UNRECOVERABLE) — retry once
        import time as _time
        _time.sleep(45)
        res = run_bass_kernel_spmd(nc, in_maps, core_ids=list(range(8)))
    parts = [res.results[c]["out"] for c in range(8)]
    full = np.stack([
        parts[0] + parts[1] + parts[2] + parts[3],
        parts[4] + parts[5] + parts[6] + parts[7],
    ]).astype(np.float32)
    return full


# revision 5
# speedup vs baseline: 1.0201x; 1.0013x over previous
"""Bass/Tile TRN2 kernel v5 for CrossAttention (B=2, N=4096, D=512, H=8, DH=64).

Sharding: batch*heads over 8 cores — core c handles batch c//4 and heads
(c%4)*2, (c%4)*2+1.

Design (cost-model driven):
  - S = K^T Q in fp8e4 DoubleRow (0.5 cycles/row). The two DoubleRow k-tiles
    carry (Q_hi, Q_lo) — a double-fp8 split of Q — against duplicated fp8 K,
    so only K's fp8 rounding reaches the logits (~1e-2 Frobenius).
  - exp split across engines: 20/32 key-chunks on ACT (exp(S*SCALE)), 12/32
    via GPSIMD pow (DVE stages S to SBUF fp16, Pool computes (e^SCALE)**S).
  - O = [V|1]^T P in fp16; denominator rides as V's 65th column.
  - Projections (phase A) interleave with the first query block, sharing the
    S-tile PSUM slots; S pipeline runs with 3-chunk lookahead.
  - Per-block epilogue is split so the PE never starves: at the block
    boundary only two DVE copies drain the O accumulators (freeing their
    PSUM bank for the next block); the normalization (reciprocal, a PE
    broadcast-matmul, DVE multiplies) and the output projection run spread
    over the next block.
"""

import sys

if "/opt/trn_rl_repo" not in sys.path:
    sys.path.insert(0, "/opt/trn_rl_repo")

import numpy as np

B, N, D = 2, 4096, 512
H, DH = 8, 64
SCALE = DH ** -0.5
P = 128
IB = 512            # i/query block
NDC = D // P        # 4 contraction chunks for projections
NIB = N // IB       # 8
NJC = N // P        # 32 key chunks per query block
LA = 3              # S-matmul chunks in flight (= st pool bufs)

# key-chunk indices handled by the GPSIMD pow path (rest go to ACT exp)
def is_pool(jc):
    return jc % 3 == 1  # 11 of 32


def pool_lag(jc):
    return POOL_LAG
# extra O-emission lag for pool-path chunks (their exp chain is longer)
POOL_LAG = 4

_CACHE: dict = {}


def _build():
    import concourse.mybir as mybir
    from concourse import bacc
    from concourse.tile import TileContext

    f32 = mybir.dt.float32
    f32r = mybir.dt.float32r
    fp16 = mybir.dt.float16
    fp8 = mybir.dt.float8e4
    Exp = mybir.ActivationFunctionType.Exp
    DR = mybir.MatmulPerfMode.DoubleRow
    Pow = mybir.AluOpType.pow
    Mult = mybir.AluOpType.mult
    Sub = mybir.AluOpType.subtract

    nc = bacc.Bacc("TRN2")
    # host pre-chunks contraction-dim tiles: [128, NDC, ...]
    xT = nc.dram_tensor("xT", [P, NDC, N], fp16, kind="ExternalInput")
    wq = nc.dram_tensor("wq", [P, NDC, 2 * DH], fp16, kind="ExternalInput")
    wk = nc.dram_tensor("wk", [P, NDC, 2 * DH], fp16, kind="ExternalInput")
    wv = nc.dram_tensor("wv", [P, NDC, 2 * DH], fp16, kind="ExternalInput")
    # augmented output projection: [Wo_h0; bo | Wo_h1; 0], shape [DH+1, 2*D]
    wo = nc.dram_tensor("wo", [DH + 1, 2 * D], f32r, kind="ExternalInput")
    out = nc.dram_tensor("out", [N, D], f32, kind="ExternalOutput")

    with TileContext(nc) as tc, \
         tc.tile_pool(name="persist", bufs=1) as pp:
        xtb = [pp.tile([P, NDC, IB], fp16, name=f"xt{i}", tag=f"xt{i}")
               for i in range(NIB)]
        # Q per block: [128 (2 heads x 64 d), 2 k-tiles, 512] = (Q_hi, Q_lo)
        qtb = [pp.tile([P, 2, IB], fp8, name=f"qt{i}", tag=f"qt{i}")
               for i in range(NIB)]
        # K per block: [128, 4 chunks, 2, 128]; chunks duplicated for the
        # DoubleRow stationary pair
        ktb = [pp.tile([P, NDC, 2, P], fp8, name=f"kt{i}", tag=f"kt{i}")
               for i in range(NIB)]
        # V per block: [128 keys, 4 chunks, 2 heads, 66]; col 64 = ones
        # (66 not 65: even inner dim so the f32-bitcast memset works)
        vtb = [pp.tile([P, NDC, 2, DH + 2], fp16, name=f"vt{i}", tag=f"vt{i}")
               for i in range(NIB)]
        wq_sb = pp.tile([P, NDC, 2 * DH], fp16, name="wq_sb", tag="wq")
        wk_sb = pp.tile([P, NDC, 2 * DH], fp16, name="wk_sb", tag="wk")
        wv_sb = pp.tile([P, NDC, 2 * DH], fp16, name="wv_sb", tag="wv")
        wo_sb = pp.tile([DH + 1, 2, D], f32r, name="wo_sb", tag="wo")
        eb = pp.tile([P, 1], f32, name="eb", tag="eb")
        # normalized O operands (ones row at partition DH feeds the bias row)
        ots0 = pp.tile([DH + 1, IB], f32r, name="ots0", tag="ots0")
        ots1 = pp.tile([DH + 1, IB], f32r, name="ots1", tag="ots1")

        nc.sync.dma_start(wq_sb[:, :, :], wq[:, :, :])
        nc.sync.dma_start(wk_sb[:, :, :], wk[:, :, :])
        nc.sync.dma_start(wv_sb[:, :, :], wv[:, :, :])
        nc.sync.dma_start(xtb[0][:, :, :], xT[:, :, 0:IB])
        nc.sync.dma_start(wo_sb[:, :, :], wo[:, :])
        for b in range(1, NIB):
            nc.sync.dma_start(xtb[b][:, :, :],
                              xT[:, :, b * IB:(b + 1) * IB])
        nc.vector.memset(eb[:], float(np.exp(SCALE)))
        nc.vector.memset(ots0[DH:DH + 1, :].bitcast(f32), 1.0)
        nc.vector.memset(ots1[DH:DH + 1, :].bitcast(f32), 1.0)
        # two packed fp16 1.0 values as an f32 bit pattern
        ones_f16x2 = float(np.frombuffer(b"\x00\x3c\x00\x3c", np.float32)[0])
        for i in range(NIB):
            # whole vt = fp16 ones; V copies later overwrite cols 0:DH
            nc.vector.memset(vtb[i][:, :, :, :].bitcast(f32), ones_f16x2)

        with tc.tile_pool(name="ps", bufs=LA, space="PSUM") as ps_pool, \
             tc.tile_pool(name="po", bufs=1, space="PSUM") as po_pool, \
             tc.tile_pool(name="pt", bufs=12) as pt_pool, \
             tc.tile_pool(name="sc", bufs=8) as sc_pool, \
             tc.tile_pool(name="ep", bufs=3) as ep_pool, \
             tc.tile_pool(name="ot", bufs=5) as ot_pool:

            def phase_a(b):
                """Project block b's K, Q (-> fp8) and V (-> fp16)."""
                xt = xtb[b]
                pk = ps_pool.tile([P, NDC, P], f32, tag="st", name="pk")
                for dc in range(NDC):
                    nc.tensor.matmul(pk[:, :, :], wk_sb[:, dc, :], xt[:, dc, :],
                                     start=(dc == 0), stop=(dc == NDC - 1))
                nc.vector.tensor_copy(
                    ktb[b][:, :, :, :],
                    pk[:, :, :].unsqueeze(2).broadcast_to((P, NDC, 2, P)))
                pq = ps_pool.tile([P, IB], f32, tag="st", name="pq")
                for dc in range(NDC):
                    nc.tensor.matmul(pq[:], wq_sb[:, dc, :], xt[:, dc, :],
                                     start=(dc == 0), stop=(dc == NDC - 1))
                nc.vector.tensor_copy(qtb[b][:, 0, :], pq[:])
                # Q_lo = Q - fp8(Q), rounded to fp8 (subnormals cover it)
                with nc.allow_low_precision(reason="double-fp8 residual"):
                    nc.vector.tensor_tensor(qtb[b][:, 1, :], pq[:],
                                            qtb[b][:, 0, :], Sub)
                pv = ps_pool.tile([P, NDC, 2, DH], f32, tag="st", name="pv")
                for q in range(NDC):
                    for dc in range(NDC):
                        nc.tensor.matmul(
                            pv[:, q, :, :], xt[:, dc, q * P:(q + 1) * P],
                            wv_sb[:, dc, :],
                            start=(dc == 0), stop=(dc == NDC - 1))
                nc.vector.tensor_copy(vtb[b][:, :, :, 0:DH], pv[:, :, :, :])

            def emit_s(ib, jc):
                """S matmuls + exp for key-chunk jc of query block ib."""
                qt = qtb[ib]
                blk, c = jc // NDC, jc % NDC
                kt = ktb[blk]
                st = ps_pool.tile([P, 2 * IB], f32, tag="st", name="st")
                nc.tensor.matmul(st[:, 0:IB], kt[0:DH, c, :, :],
                                 qt[0:DH, :, :],
                                 start=True, stop=True, perf_mode=DR)
                nc.tensor.matmul(st[:, IB:2 * IB], kt[DH:P, c, :, :],
                                 qt[DH:P, :, :],
                                 start=True, stop=True, perf_mode=DR,
                                 tile_position=(64, 0))
                pt = pt_pool.tile([P, 2 * IB], fp16, tag="pt", name="pt")
                if is_pool(jc):
                    sc = sc_pool.tile([P, 2 * IB], fp16, tag="sc", name="sc")
                    nc.vector.tensor_copy(sc[:], st[:])
                    nc.gpsimd.tensor_tensor(
                        pt[:], eb[:].broadcast_to((P, 2 * IB)), sc[:], Pow)
                else:
                    nc.scalar.activation(pt[:], st[:], Exp, scale=SCALE)
                return pt

            def emit_o(o0, o1, jc, pt, first, last):
                blk, c = jc // NDC, jc % NDC
                vt = vtb[blk]
                nc.tensor.matmul(o0[:], vt[:, c, 0, 0:DH + 1], pt[:, 0:IB],
                                 start=first, stop=last)
                nc.tensor.matmul(o1[:], vt[:, c, 1, 0:DH + 1], pt[:, IB:2 * IB],
                                 start=first, stop=last)

            def epi_norm(o0, o1):
                """Drain O accumulators to SBUF (frees their PSUM bank) and
                compute the denominators' reciprocals."""
                oc0 = ep_pool.tile([DH + 1, IB], f32, tag="oc0", name="oc0")
                oc1 = ep_pool.tile([DH + 1, IB], f32, tag="oc1", name="oc1")
                nc.vector.tensor_copy(oc0[:], o0[:])
                nc.vector.tensor_copy(oc1[:], o1[:])
                rv0 = ep_pool.tile([1, IB], f32, tag="rv0", name="rv0")
                rv1 = ep_pool.tile([1, IB], f32, tag="rv1", name="rv1")
                nc.vector.reciprocal(rv0[:], oc0[DH:DH + 1, :])
                nc.vector.reciprocal(rv1[:], oc1[DH:DH + 1, :])
                return oc0, oc1, (rv0, rv1)

            def epi_scale(oc0, oc1, rv):
                """ots_h = O_h * (1/denom) broadcast per query."""
                rv0, rv1 = rv
                rb0 = ep_pool.tile([DH, IB], f32, tag="rb0", name="rb0")
                rb1 = ep_pool.tile([DH, IB], f32, tag="rb1", name="rb1")
                nc.gpsimd.partition_broadcast(rb0[:], rv0[:])
                nc.gpsimd.partition_broadcast(rb1[:], rv1[:])
                nc.vector.tensor_tensor(ots0[0:DH, :], oc0[0:DH, :],
                                        rb0[:], Mult)
                nc.vector.tensor_tensor(ots1[0:DH, :], oc1[0:DH, :],
                                        rb1[:], Mult)

            def epi_proj(ib, q):
                """Output projection of one 128-query chunk."""
                ppx = ps_pool.tile([P, D], f32, tag="st", name="ppx")
                q0, q1 = q * P, (q + 1) * P
                nc.tensor.matmul(ppx[:, :], ots0[:, q0:q1],
                                 wo_sb[:, 0, :], start=True, stop=False)
                nc.tensor.matmul(ppx[:, :], ots1[:, q0:q1],
                                 wo_sb[:, 1, :], start=False, stop=True)
                otile = ot_pool.tile([P, D], f32, tag="ot", name="otile")
                nc.vector.tensor_copy(otile[:, :], ppx[:, :])
                r0 = ib * IB + q * P
                nc.sync.dma_start(out[r0:r0 + P, :], otile[:, :])

            # O-emission order: pool-path chunks drop POOL_LAG positions
            # later so the PE never waits on their longer exp chain
            o_order = sorted(range(NJC),
                             key=lambda c: (c + (pool_lag(c) if is_pool(c)
                                                 else 0), c))
            phase_a(0)
            phase_a(1)
            pend = None  # (ib, oc0, oc1, rv) awaiting scale+projection
            pts = {}
            o0 = o1 = None
            NG = NIB * NJC
            for j in range(LA):
                pts[j] = emit_s(j // NJC, j % NJC)
            for g in range(NG):
                ib, k = g // NJC, g % NJC
                if k == 0:
                    if ib > 0:
                        oc0, oc1, rv = epi_norm(o0, o1)
                        pend = (ib - 1, oc0, oc1, rv)
                    o0 = po_pool.tile([DH + 1, IB], f32, tag="o0", name="o0")
                    o1 = po_pool.tile([DH + 1, IB], f32, tag="o1", name="o1")
                if g + LA < NG:
                    gs = g + LA
                    pts[gs] = emit_s(gs // NJC, gs % NJC)
                c = o_order[k]
                emit_o(o0, o1, c, pts.pop(ib * NJC + c),
                       first=(k == 0), last=(k == NJC - 1))
                if ib == 0:
                    if k >= 2 and (k - 2) % 4 == 0 and (k - 2) // 4 + 2 < NIB:
                        phase_a((k - 2) // 4 + 2)
                elif pend is not None:
                    if k == 2:
                        epi_scale(pend[1], pend[2], pend[3])
                    elif k in (5, 7, 9, 11):
                        epi_proj(pend[0], (k - 5) // 2)
                        if k == 11:
                            pend = None
            oc0, oc1, rv = epi_norm(o0, o1)
            epi_scale(oc0, oc1, rv)
            for q in range(NDC):
                epi_proj(NIB - 1, q)

    nc.compile()
    return nc


def _get_nc():
    if "nc" not in _CACHE:
        _CACHE["nc"] = _build()
    return _CACHE["nc"]


def kernel(x, Wq, Wk, Wv, Wo, bo):
    from concourse.bass_utils import run_bass_kernel_spmd

    x = np.asarray(x, dtype=np.float32)
    Wq = np.asarray(Wq, dtype=np.float32)
    Wk = np.asarray(Wk, dtype=np.float32)
    Wv = np.asarray(Wv, dtype=np.float32)
    Wo = np.asarray(Wo, dtype=np.float32)
    bo = np.asarray(bo, dtype=np.float32)

    nc = _get_nc()

    def prechunk(a):  # [512, M] -> [128, 4, M] with row r = dc*128+p
        return np.ascontiguousarray(
            a.reshape(NDC, P, a.shape[1]).transpose(1, 0, 2)).astype(np.float16)

    xTs = [prechunk(x[b].T) for b in range(B)]
    in_maps = []
    for c in range(8):
        b, p = c // 4, c % 4
        sl = slice(p * 2 * DH, (p + 1) * 2 * DH)
        wo_aug = np.zeros((DH + 1, 2 * D), dtype=np.float32)
        wo_aug[0:DH, 0:D] = Wo[sl, :][0:DH, :]
        wo_aug[0:DH, D:2 * D] = Wo[sl, :][DH:2 * DH, :]
        if p == 0:
            wo_aug[DH, 0:D] = bo
        in_maps.append({
            "xT": xTs[b],
            "wq": prechunk(Wq[:, sl]),
            "wk": prechunk(Wk[:, sl]),
            "wv": prechunk(Wv[:, sl]),
            "wo": wo_aug,
        })

    try:
        res = run_bass_kernel_spmd(nc, in_maps, core_ids=list(range(8)))
    except Exception:
        # transient device wedge (NRT_EXEC_UNIT_UNRECOVERABLE) — retry once
        import time as _time
        _time.sleep(45)
        res = run_bass_kernel_spmd(nc, in_maps, core_ids=list(range(8)))
    parts = [res.results[c]["out"] for c in range(8)]
    full = np.stack([
        parts[0] + parts[1] + parts[2] + parts[3],
        parts[4] + parts[5] + parts[6] + parts[7],
    ]).astype(np.float32)
    return full
